# revision 8
# baseline (speedup 1.0000x reference)
"""Trainium2 Bass kernel for a binarized ResNet BasicBlock (stride-2).

Reference computation (per image):
    residual = BN2(conv1x1(avgpool2x2(x), w_ds))          # full precision
    body     = BN1(conv3x3_s2_p1(sign(x), sign(w_body)))  # binarized
    out      = body + residual

Shapes: x [16, 32, 224, 224] f32 -> out [16, 64, 112, 112] f32.
Sharding: data-parallel over batch, 2 images per core on 8 cores.

v4 layout (all-fp8, all-DoubleRow). Per chunk PAIR (two vertically adjacent
8-output-row chunks of one image; the even chunk's rows live on SBUF
partitions 0:64, the odd chunk's on 64:128, feeding the two PE row-group
strips concurrently):
  * Host pre-casts the input to fp8e4 (sign bit preserved; the residual
    path tolerates the quantization, |err| ~1e-2 vs tolerance ~2) and
    splits each row into [even columns (112) | odd columns (112)]. One fp8
    DMA per pair loads V [128, 9, 224].
  * S holds sign(x) as +-1 fp8 in the same split-column layout, slot
    stride 240: [pad(2) | even 2:114 | pad 114:116 | odd 116:228]. Two DVE
    tensor_scalar ops per pair compute (v & 0x8080) | 0x3838 on uint16
    views (keeps the DVE 2x packed mode). Pad bytes are zeroed once per
    physical buffer; the kx=0 tap at X=0 reads byte 115.
  * Body matmuls are fp8 DoubleRow: rhs is a custom 4D AP
    [K=64, Ko=2, rows=4, cols=112] where Ko and rows both stride one slot,
    so output row y reads slots (y, y+1): par0 cells see (row 2Y-2, 2Y) ->
    weights (0, w_ky1); par1 cells see (2Y-1, 2Y+1) -> (w_ky0, w_ky2). One
    DR matmul per (kx, 4-row group) covers all three ky taps.
  * Residual matmuls are DoubleRow as well: Ko pairs (even col X, odd col
    X) at step 112 with weights (wr, wr) compute the 2x2-pool 1x1-conv
    column sum; rows stride one V slot. One DR matmul per 4-row group,
    weights pre-scaled by inv2/(4*inv1).
  * DoubleRow requires output column group 0, so BOTH halves write PSUM
    partitions 0:64 (row-tiled, tile_position (0,0)/(64,0)) into one
    shared PSUM tile [64, 4, 512] per pair (t-slices 2q+t); 2 such tiles
    double-buffer into the 8 banks.
  * One ScalarE activation per pair (Identity, scale/bias vectors) applies
    both BNs evacuating PSUM->SBUF f32 [64, 16, 112], and one DMA per pair
    (SP / Activation queues alternating) stores 16 contiguous output rows.
"""

import numpy as np
import ml_dtypes

EPS = 1e-5

# Full-problem constants (hardcoded; the harness provides only kernel.py).
B, CIN, COUT, H, W = 16, 32, 64, 224, 224
N_CORES = 8
B_CORE = B // N_CORES  # 2 images per core

CHUNK_ROWS = 8
SPAD = 240   # padded S row-slot stride (fp8 bytes), %16 == 0 for DoubleRow
SEVEN = 2    # S even-column block byte offset
SODD = 116   # S odd-column block byte offset (kx=0 pad byte at 115)


def build_nc(b_core=B_CORE, cin=CIN, cout=COUT, h=H, w=W,
             chunk_rows=CHUNK_ROWS, loop_reps=1, ablate=None):
    """Build the Bass program for one core processing b_core images.

    loop_reps > 1 wraps the whole computation in a hardware loop (identical
    results each iteration) — used only for wall-clock timing amplification.
    """
    from contextlib import nullcontext
    import concourse.bass as bass
    import concourse.bacc as bacc
    import concourse.mybir as mybir
    import concourse.tile as tile

    ho, wo = h // 2, w // 2
    assert ho % chunk_rows == 0
    n_chunks = ho // chunk_rows
    assert chunk_rows % 4 == 0
    T = chunk_rows // 4  # 4 output rows per matmul tile
    nslots = chunk_rows + 1  # one extra leading row slot per chunk

    f32 = mybir.dt.float32
    fp8 = mybir.dt.float8e4
    u16 = mybir.dt.uint16
    DR = mybir.MatmulPerfMode.DoubleRow

    nc = bacc.Bacc("TRN2", target_bir_lowering=False, debug=False)

    # Input is pre-arranged on the host as one payload per chunk PAIR:
    # zz[pair, p, slot, u] fp8, partitions 0:64 = even chunk's rows
    # ((par, ci) major, slot = leading-row + 8 rows, u = even|odd column
    # split), 64:128 = odd chunk's.
    n_pairs = (b_core * n_chunks + 1) // 2
    zz = nc.dram_tensor("zz", [n_pairs, 128, nslots, w], fp8,
                        kind="ExternalInput")
    # DoubleRow body weights [p, kx, ko, co]; partitions 64:128 duplicate
    # 0:64 so each PE row group loads from its own partition half.
    w_dr = nc.dram_tensor("w_dr", [128, 3, 2, cout], fp8, kind="ExternalInput")
    w_res = nc.dram_tensor("w_res", [128, 2, cout], fp8, kind="ExternalInput")
    bn_sb = nc.dram_tensor("bn_sb", [cout, 2], f32, kind="ExternalInput")
    out = nc.dram_tensor("out", [b_core, cout, ho, wo], f32,
                         kind="ExternalOutput")

    def window_ap(base, ko_step, nrows, row_step):
        # [K=64, Ko=2, rows, cols] built from a [K, 1 or 2, cols] slice.
        return bass.AP(base.tensor, base.offset,
                       [list(base.ap[0]), [ko_step, 2], [row_step, nrows],
                        list(base.ap[-1])])

    with tile.TileContext(nc) as tc:
        with tc.tile_pool(name="consts", bufs=1) as cpool:
            wdr = cpool.tile([128, 3, 2, cout], fp8)
            nc.sync.dma_start(out=wdr[:, :, :, :], in_=w_dr.ap()[:, :, :, :])
            wrd = cpool.tile([128, 2, cout], fp8)
            nc.scalar.dma_start(out=wrd[:, :, :], in_=w_res.ap()[:, :, :])
            sb_ = cpool.tile([cout, 2], f32)
            nc.sync.dma_start(out=sb_[:, :], in_=bn_sb.ap()[:, :])
            sc, bi = sb_[:, 0:1], sb_[:, 1:2]

            with (
                tc.tile_pool(name="vpool", bufs=4) as vpool,
                tc.tile_pool(name="spool", bufs=1) as spool,
                tc.tile_pool(name="opool", bufs=4) as opool,
                tc.tile_pool(name="pspool", bufs=2, space="PSUM") as pspool,
            ):
                # S buffers are managed manually (not pool-cycled) so their
                # zero-pad bytes can be initialized exactly once; sign
                # writes never touch them afterwards.
                n_sbufs = 3
                s_bufs = []
                for si in range(n_sbufs):
                    sb = spool.tile([128, nslots, SPAD], fp8, name=f"sbuf{si}")
                    nc.vector.memset(sb[:, :, 0:SEVEN], 0.0)
                    nc.vector.memset(sb[:, :, SEVEN + w // 2 : SODD], 0.0)
                    s_bufs.append(sb)
                # Dedicated buffer for pairs whose even chunk is c == 0: its
                # q0 slot 0 is the conv's zero padding row (the sign op would
                # turn DMA'd zeros into +1s, so it must never write there).
                sb0 = spool.tile([128, nslots, SPAD], fp8, name="sbufc0")
                nc.vector.memset(sb0[:, :, 0:SEVEN], 0.0)
                nc.vector.memset(sb0[:, :, SEVEN + w // 2 : SODD], 0.0)
                nc.vector.memset(sb0[0:64, 0:1, :], 0.0)

                reps_ctx = (
                    tc.For_i(0, loop_reps, 1) if loop_reps > 1 else nullcontext()
                )
                G = b_core * n_chunks
                with reps_ctx:
                  for pair in range(n_pairs):
                    halves = [q for q in range(2) if 2 * pair + q < G]
                    c0_pair = (2 * pair) % n_chunks == 0
                    v = vpool.tile([128, nslots, w], fp8)
                    s = sb0 if c0_pair else s_bufs[pair % n_sbufs]
                    if ablate != "no_in":
                        nc.gpsimd.dma_start(out=v[:, :, :],
                                            in_=zz.ap()[pair, :, :, :])
                        # sign bits: s = (v & 0x8080) | 0x3838 (+-1 fp8), on
                        # u16 views; one op per column-parity block. For a
                        # c == 0 pair, q0's slot 0 (padding) must stay zero.
                        for plo, phi, jlo in (
                            [(0, 64, 1), (64, 128, 0)] if c0_pair
                            else [(0, 128, 0)]
                        ):
                            for so, vo in ((SEVEN, 0), (SODD, w // 2)):
                                nc.vector.tensor_scalar(
                                    s.bitcast(u16)[plo:phi, jlo:,
                                                   so // 2 : (so + w // 2) // 2],
                                    v.bitcast(u16)[plo:phi, jlo:,
                                                   vo // 2 : (vo + w // 2) // 2],
                                    0x8080,
                                    0x3838,
                                    mybir.AluOpType.bitwise_and,
                                    mybir.AluOpType.bitwise_or,
                                )
                    ps = pspool.tile([64, 2 * T, 512], f32, tag="ps")
                    if ablate != "io_only":
                        for kx, so in ((0, SODD - 1), (1, SEVEN), (2, SODD)):
                            for t in range(T):
                                for q in halves:
                                    p0 = 64 * q
                                    base = s[p0 : p0 + 64, 4 * t : 4 * t + 2,
                                             so : so + wo]
                                    nc.tensor.matmul(
                                        ps[0:64, 2 * q + t, 0 : 4 * wo],
                                        wdr[p0 : p0 + 64, kx, :, :],
                                        window_ap(base, SPAD, 4, SPAD),
                                        start=(kx == 0), stop=False,
                                        perf_mode=DR,
                                        tile_position=(p0, 0),
                                    )
                        for t in range(T):
                            j0 = 1 + 4 * t
                            for q in halves:
                                p0 = 64 * q
                                base = v[p0 : p0 + 64, j0 : j0 + 1, 0:wo]
                                nc.tensor.matmul(
                                    ps[0:64, 2 * q + t, 0 : 4 * wo],
                                    wrd[p0 : p0 + 64, :, :],
                                    window_ap(base, wo, 4, w),
                                    start=False, stop=True,
                                    perf_mode=DR,
                                    tile_position=(p0, 0),
                                )
                        g0 = 2 * pair
                        b, c = divmod(g0, n_chunks)
                        y0 = c * chunk_rows
                        nrows = chunk_rows * len(halves)
                        o = opool.tile([64, 2 * chunk_rows, wo], f32)
                        # BN + evacuate: out = psum*inv1 + (shift1+shift2)
                        nc.scalar.activation(
                            o.rearrange("p (g j) x -> p g (j x)", g=2 * T),
                            ps[:, :, 0 : 4 * wo],
                            mybir.ActivationFunctionType.Identity,
                            bias=bi,
                            scale=sc,
                        )
                        out_eng = nc.sync if pair % 2 == 0 else nc.scalar
                        out_eng.dma_start(
                            out=out.ap()[b, :, y0 : y0 + nrows, :],
                            in_=o[:, 0:nrows, :],
                        )
    nc.compile()
    return nc


def prep_weights(w_body, w_ds, bn1_gamma, bn1_beta, bn1_mean, bn1_var,
                 bn2_gamma, bn2_beta, bn2_mean, bn2_var):
    """Host-side parameter folding (all small tensors)."""
    fp8 = ml_dtypes.float8_e4m3
    cout, cin = w_body.shape[0], w_body.shape[1]
    inv1 = (bn1_gamma / np.sqrt(bn1_var + EPS)).astype(np.float32)
    inv2 = (bn2_gamma / np.sqrt(bn2_var + EPS)).astype(np.float32)
    shift1 = (bn1_beta - bn1_mean * inv1).astype(np.float32)
    shift2 = (bn2_beta - bn2_mean * inv2).astype(np.float32)

    wb_sign = np.where(w_body >= 0, 1.0, -1.0).astype(np.float32)  # [co,ci,ky,kx]

    # DoubleRow body weights [p, kx, ko, co]: par0 rows hold (0, w_ky1)
    # (slot j is row 2Y-2, unwanted), par1 rows hold (w_ky0, w_ky2).
    wdr = np.zeros((128, 3, 2, cout), np.float32)
    for kx in range(3):
        wdr[0:cin, kx, 1] = wb_sign[:, :, 1, kx].T          # par0, ko=1: ky1
        wdr[cin : 2 * cin, kx, 0] = wb_sign[:, :, 0, kx].T  # par1, ko=0: ky0
        wdr[cin : 2 * cin, kx, 1] = wb_sign[:, :, 2, kx].T  # par1, ko=1: ky2
    wdr[64:128] = wdr[0:64]

    # Residual weights with BN2 folded and divided by BN1 scale (the final
    # activation multiplies everything by inv1); identical on both Ko lanes
    # (even + odd column of the 2x2 pool).
    wres = w_ds[:, :, 0, 0] * (inv2 / (4.0 * inv1))[:, None]  # [co, ci]
    w_res = np.tile(wres.T[:, None, :], (4, 2, 1)).reshape(128, 2, cout)

    return dict(
        w_dr=wdr.astype(fp8),
        w_res=w_res.astype(fp8),
        bn_sb=np.stack([inv1, shift1 + shift2], axis=1),
    )


def make_zz(x, cin=CIN, h=H, w=W, chunk_rows=CHUNK_ROWS):
    """Host layout prep: per-chunk-pair fp8 DMA payloads.

    x: [b, ci, r, u] f32. Returns zz[pair, p, slot, u] fp8 where partition
    p = 64*(chunk parity) + par*ci-major, slot j holds input row
    2*(chunk_rows*c - 1 + j) + par split as [even cols | odd cols]; the
    leading slot of chunk 0 is zero padding.
    """
    b_core = x.shape[0]
    hh = h // 2
    n_chunks = hh // chunk_rows
    ns = chunk_rows + 1
    xv = x.reshape(b_core, cin, hh, 2, w // 2, 2).transpose(0, 3, 1, 2, 5, 4)
    # xv: [b, par, ci, r2, colpar, u'] -> rows split into even|odd columns
    xv = xv.reshape(b_core, 2 * cin, hh, w).astype(ml_dtypes.float8_e4m3)
    G = b_core * n_chunks
    zz = np.zeros(((G + 1) // 2, 128, ns, w), ml_dtypes.float8_e4m3)
    for g in range(G):
        b, c = divmod(g, n_chunks)
        q, y0 = g % 2, c * chunk_rows
        jlo = 1 if c == 0 else 0
        zz[g // 2, 64 * q : 64 * q + 64, jlo:ns] = xv[
            b, :, y0 - 1 + jlo : y0 + chunk_rows, :]
    return zz


def kernel(x, w_body, bn1_gamma, bn1_beta, bn1_mean, bn1_var,
           w_ds, bn2_gamma, bn2_beta, bn2_mean, bn2_var):
    from concourse.bass_utils import run_bass_kernel_spmd

    x = np.asarray(x, dtype=np.float32)
    params = prep_weights(
        np.asarray(w_body, np.float32), np.asarray(w_ds, np.float32),
        np.asarray(bn1_gamma, np.float32), np.asarray(bn1_beta, np.float32),
        np.asarray(bn1_mean, np.float32), np.asarray(bn1_var, np.float32),
        np.asarray(bn2_gamma, np.float32), np.asarray(bn2_beta, np.float32),
        np.asarray(bn2_mean, np.float32), np.asarray(bn2_var, np.float32),
    )

    nc = build_nc()
    in_maps = [
        {"zz": make_zz(x[k * B_CORE : (k + 1) * B_CORE]), **params}
        for k in range(N_CORES)
    ]
    res = run_bass_kernel_spmd(nc, in_maps, core_ids=list(range(N_CORES)))
    return np.concatenate([r["out"] for r in res.results], axis=0)
